# revision 38
# baseline (speedup 1.0000x reference)
"""Trainium2 Bass kernel for ConvDownsample2d (FIR blur + 3x3/s2 conv + bias + leaky_relu*sqrt2).

Contract: kernel(**inputs) takes FULL inputs (x[16,512,64,64] f32, weight[512,512,3,3],
bias[512], fir[4,4]) and returns the FULL output [16,512,32,32] f32.

Strategy (hardcoded for this problem size):
  - Data-parallel over batch: 16 images / 8 cores = 2 images per core. No collectives.
  - Column-phase decomposition: host supplies 4 fp16 plane tensors per image
    (even/odd columns of x, each also at a one-element-shifted SBUF offset) so
    every VectorE blur operand is 4B-aligned.
  - Row-blur factoring: [1,3,3,1] = [1,1] (*) [1,2,1]. The [1,1] half is folded
    into the conv weights on host (12 row x col taps, w'0=w0, w'1=w0+w1,
    w'2=w1+w2, w'3=w2), so the device only computes the [1,2,1] row blur.
  - Blur: z_e[v] = (x_e[v-1]+x_o[v]) + 3(x_o[v-1]+x_e[v]),
          z_o[v] = (x_o[v-1]+x_e[v+1]) + 3(x_e[v]+x_o[v]) per column phase,
    then y'_p[r] = (z_p[r-1] + z_p[r+1]) + 2*z_p[r], split by row parity so
    every TensorE rhs is a fully-contiguous slab (HW: ~104 ns/MM packed vs
    ~201 ns/MM for the naive stride-2 rhs at N=512 fp16).
    Pairwise adds on VectorE (fp16 2x mode), the x3/x2 via ScalarE scaled copies.
  - Conv: accumulated 128x128x512 fp16 matmuls (channels on partitions, 12 taps
    x 4 cin-chunks into PSUM); mc-outer order staggers PSUM completion so the
    Prelu epilogue drains overlap the next output-chunk's matmuls.
  - Epilogue: single ScalarE Prelu (bias + leaky_relu(0.2)) per PSUM tile,
    DMA out f32 (sqrt2 folded into weights+bias on host).
"""

import sys

for p in ("/opt/trn_rl_repo", "/opt/pypackages"):
    if p not in sys.path:
        sys.path.insert(0, p)

import numpy as np
from contextlib import ExitStack

from concourse import bass, bacc, mybir, tile
from concourse.bass_utils import run_bass_kernel_spmd

F16 = mybir.dt.float16
F32 = mybir.dt.float32

NCORES = 8
NPC = 2            # images per core
N_TOT = 16         # total batch
CIN = 512
COUT = 512
H = W = 64
OH = OW = 32
KS = 3
W_LRMUL = 1.0 / np.sqrt(CIN * COUT * KS * KS)
SQRT2 = np.sqrt(2.0)

MT = 4             # top margin of z tiles (rows)
PW = 36            # plane width (cols): interior v=0..31 at cols 2..33
XB = 2             # x-plane prefetch buffers
YB = 3             # y buffers

USE_PRELU = True   # fused ScalarE Prelu epilogue (not implemented in CoreSim;
                   # validated end-to-end on HW via the rel-err check)

_CACHE = {}


def _build(reps=1):
    nc = bacc.Bacc("TRN2", target_bir_lowering=False, debug=False, enable_asserts=False)

    xe_d = nc.dram_tensor("xe", [NPC, CIN, H, PW], F16, kind="ExternalInput")
    xes_d = nc.dram_tensor("xes", [NPC, CIN, H, PW], F16, kind="ExternalInput")
    xo_d = nc.dram_tensor("xo", [NPC, CIN, H, PW], F16, kind="ExternalInput")
    xos_d = nc.dram_tensor("xos", [NPC, CIN, H, PW], F16, kind="ExternalInput")
    w_d = nc.dram_tensor("w", [CIN, 12, COUT], F16, kind="ExternalInput")
    b_d = nc.dram_tensor("b", [128, 4], F32, kind="ExternalInput")
    o_d = nc.dram_tensor("out", [NPC, COUT, OH, OW], F32, kind="ExternalOutput")

    AL = mybir.AluOpType
    AF = mybir.ActivationFunctionType

    with tile.TileContext(nc) as tc, ExitStack() as ctx:
        cpool = ctx.enter_context(tc.tile_pool(name="const", bufs=1))
        bpool = ctx.enter_context(tc.tile_pool(name="blur", bufs=1))
        opool = ctx.enter_context(tc.tile_pool(name="outp", bufs=8))
        ppool = ctx.enter_context(
            tc.tile_pool(name="psum", bufs=1, space=bass.MemorySpace.PSUM)
        )

        # --- constants (kc=0 weights first so the first matmuls are not stuck
        # behind the full weight transfer) ---
        w_sb = cpool.tile([128, 4, 12, COUT], F16, name="w_sb")
        nc.sync.dma_start(out=w_sb[:, 0], in_=w_d[0:128])
        b_sb = cpool.tile([128, 4], F32, name="b_sb")
        nc.sync.dma_start(out=b_sb[:], in_=b_d[:])

        # --- blur tiles ---
        def planes(name, n, rows=H):
            return [bpool.tile([128, rows, PW], F16, name=f"{name}{i}") for i in range(n)]

        xe = planes("xe", XB)
        xes = planes("xes", XB)
        xo = planes("xo", XB)
        xos = planes("xos", XB)
        ta = planes("ta", 2)
        tb = planes("tb", 2)
        b3 = planes("b3", 2)
        z2 = planes("z2", 2, rows=66)    # 2*z over z rows -1..63
        tE = planes("tE", 1, rows=32)[0]
        tO = planes("tO", 1, rows=34)[0]
        ze = bpool.tile([128, 70, PW], F16, name="ze")
        zo = bpool.tile([128, 70, PW], F16, name="zo")
        # y' planes [36, PW]: row-phase E: u at row u+2 (guard row 1 = u=-1);
        # row-phase O: u at row u+2, computed from u=-1 (row 1)
        yEe = planes("yEe", YB, rows=36)
        yEo = planes("yEo", YB, rows=36)
        yOe = planes("yOe", YB, rows=36)
        yOo = planes("yOo", YB, rows=36)

        # --- zero guards once; per-chunk writes stay in the interior ---
        for z_ in (ze, zo):
            nc.vector.memzero(z_[:, 2:4, 2:34])            # z rows -2, -1
            nc.vector.memzero(z_[:, 68:69, 2:34])          # z row 64
        for y_ in yEe + yEo:
            nc.vector.memzero(y_[:, 1:2, 0:PW])            # E[-1] = y'[-2] = 0
        for y_ in yEo + yOo:
            nc.vector.memzero(y_[:, 0:36, 0:2])            # col v=-1 (odd col phase)

        n_imgs = reps * NPC
        chunks = [(i % NPC, kc) for i in range(n_imgs) for kc in range(4)]

        def dma_planes(g):
            n, kc = chunks[g]
            sl = slice(kc * 128, (kc + 1) * 128)
            i = g % XB
            nc.sync.dma_start(out=xe[i][:], in_=xe_d[n, sl])
            nc.sync.dma_start(out=xo[i][:], in_=xo_d[n, sl])
            nc.sync.dma_start(out=xes[i][:], in_=xes_d[n, sl])
            nc.sync.dma_start(out=xos[i][:], in_=xos_d[n, sl])

        for g in range(min(XB, len(chunks))):
            dma_planes(g)
        for kc in range(1, 4):
            nc.sync.dma_start(out=w_sb[:, kc], in_=w_d[kc * 128:(kc + 1) * 128])

        C = slice(2, 34)       # interior plane cols (v = col-2)
        for g, (n, kc) in enumerate(chunks):
            i = g % XB
            xe_, xes_, xo_, xos_ = xe[i], xes[i], xo[i], xos[i]
            ta_, tb_, b3_ = ta[g % 2], tb[g % 2], b3[g % 2]

            # ---- W-blur (column phases) ----
            # z_e = (x_e[v-1] + x_o[v]) + 3*(x_o[v-1] + x_e[v])
            nc.vector.tensor_tensor(ta_[:, :, C], xes_[:, :, C], xo_[:, :, C], AL.add)
            nc.vector.tensor_tensor(tb_[:, :, C], xos_[:, :, C], xe_[:, :, C], AL.add)
            nc.scalar.activation(b3_[:, :, C], tb_[:, :, C], AF.Identity, scale=3.0)
            nc.vector.tensor_tensor(
                ze[:, MT:MT + 64, C], ta_[:, :, C], b3_[:, :, C], AL.add
            )
            # z_o = (x_o[v-1] + x_e[v+1]) + 3*(x_e[v] + x_o[v])
            ta2_, tb2_, b32_ = ta[(g + 1) % 2], tb[(g + 1) % 2], b3[(g + 1) % 2]
            nc.vector.tensor_tensor(
                ta2_[:, :, C], xos_[:, :, C], xes_[:, :, 4:36], AL.add
            )
            nc.vector.tensor_tensor(tb2_[:, :, C], xe_[:, :, C], xo_[:, :, C], AL.add)
            nc.scalar.activation(b32_[:, :, C], tb2_[:, :, C], AF.Identity, scale=3.0)
            nc.vector.tensor_tensor(
                zo[:, MT:MT + 64, C], ta2_[:, :, C], b32_[:, :, C], AL.add
            )
            # prefetch AFTER the last reader of this chunk's x planes
            if g + XB < len(chunks):
                dma_planes(g + XB)

            # ---- [1,2,1] row blur, split by row parity, per column phase ----
            # y'[r] = (z[r-1] + z[r+1]) + 2*z[r]
            # E: y'[2u] = (z[2u-1]+z[2u+1]) + 2*z[2u],        u in 0..31
            # O: y'[2u+1] = (z[2u]+z[2u+2]) + 2*z[2u+1],      u in -1..31
            ypl = ((yEe[g % YB], yOe[g % YB]), (yEo[g % YB], yOo[g % YB]))
            for pb, z_ in enumerate((ze, zo)):
                yE_, yO_ = ypl[pb]
                z2_ = z2[pb]
                # z2[j] = 2*z[j-1] for j=0..64  (z tile rows 3..67)
                nc.scalar.activation(
                    z2_[:, 0:65, C], z_[:, 3:68, C], AF.Identity, scale=2.0
                )
                nc.vector.tensor_tensor(
                    tE[:, :, C], z_[:, 3:66:2, C], z_[:, 5:68:2, C], AL.add
                )
                nc.vector.tensor_tensor(
                    yE_[:, 2:34, C], tE[:, :, C], z2_[:, 1:64:2, C], AL.add
                )
                nc.vector.tensor_tensor(
                    tO[:, 0:33, C], z_[:, 2:67:2, C], z_[:, 4:69:2, C], AL.add
                )
                nc.vector.tensor_tensor(
                    yO_[:, 1:34, C], tO[:, 0:33, C], z2_[:, 0:65:2, C], AL.add
                )

            # ---- conv taps: psum[mc][uh] += w'[i',q].T @ y'[...] ----
            # row tap i': 0 -> E[u-1], 1 -> O[u-1], 2 -> E[u], 3 -> O[u]
            # col tap q:  0 -> odd plane, v-1 (t0=1); 1 -> even, v (t0=2); 2 -> odd, v (t0=2)
            if kc == 0:
                psum = [
                    [
                        ppool.tile([128, 16, OW], F32, tag=f"ps{mc}{uh}", name=f"ps{mc}{uh}")
                        for uh in range(2)
                    ]
                    for mc in range(4)
                ]
            for mc in range(4):
                for t in range(12):
                    ip, q = divmod(t, 3)
                    lhsT = w_sb[:, kc, t, mc * 128:(mc + 1) * 128]
                    if q == 1:
                        pe_, po_ = yEe[g % YB], yOe[g % YB]
                        t0 = 2
                    else:
                        pe_, po_ = yEo[g % YB], yOo[g % YB]
                        t0 = 1 if q == 0 else 2
                    plane = pe_ if ip % 2 == 0 else po_
                    ds = 1 if ip < 2 else 2      # u-1 -> row u+1 ; u -> row u+2
                    for uh in range(2):
                        s0 = 16 * uh + ds
                        rhs = plane[:, s0:s0 + 16, t0:t0 + 32]
                        nc.tensor.matmul(
                            psum[mc][uh][:],
                            lhsT,
                            rhs,
                            start=(kc == 0 and t == 0),
                            stop=(kc == 3 and t == 11),
                        )
                if kc == 3:
                    # epilogue for this mc: overlaps the next mc's matmuls
                    with tc.high_priority():
                        for uh in range(2):
                            ob = opool.tile([128, 16, OW], F32, tag="ob", name="ob")
                            if USE_PRELU:
                                nc.scalar.activation(
                                    ob[:], psum[mc][uh][:], AF.Prelu,
                                    bias=b_sb[:, mc:mc + 1], scale=1.0, alpha=0.2,
                                )
                            else:
                                tbo = opool.tile([128, 16, OW], F32, tag="tbo", name="tbo")
                                nc.scalar.activation(
                                    tbo[:], psum[mc][uh][:], AF.Identity,
                                    bias=b_sb[:, mc:mc + 1], scale=1.0,
                                )
                                nc.vector.scalar_tensor_tensor(
                                    ob[:], tbo[:], 0.2, tbo[:], AL.mult, AL.max
                                )
                            nc.sync.dma_start(
                                out=o_d[n, mc * 128:(mc + 1) * 128, uh * 16:(uh + 1) * 16, :],
                                in_=ob[:],
                            )

    nc.compile()
    return nc


def get_nc(reps=1):
    key = f"nc{reps}"
    if key not in _CACHE:
        _CACHE[key] = _build(reps)
    return _CACHE[key]


def prep_inputs(x, weight, bias, fir):
    """Host-side shard + fold constants + phase-plane layout. Returns per-core maps."""
    x = np.asarray(x, dtype=np.float32)
    weight = np.asarray(weight, dtype=np.float32)
    bias = np.asarray(bias, dtype=np.float32)
    fir = np.asarray(fir, dtype=np.float32)

    # normalized separable fir = fir[0,0] * outer([1,3,3,1],[1,3,3,1]);
    # fold fir[0,0] into x, integer taps run on device.
    scale = float(fir[0, 0])
    x16 = (x * scale).astype(np.float16)
    x_e = x16[:, :, :, 0::2]   # [N, C, 64, 32]
    x_o = x16[:, :, :, 1::2]

    # plane layouts (width PW=36): col t <-> v = t-2 (normal) / v = t-3 (shifted);
    # guard zeros baked in.
    def lay(plane, off):
        buf = np.zeros((N_TOT, CIN, H, PW), dtype=np.float16)
        buf[:, :, :, off:off + 32] = plane
        return buf

    xe_dev = lay(x_e, 2)
    xes_dev = lay(x_e, 3)
    xo_dev = lay(x_o, 2)
    xos_dev = lay(x_o, 3)

    # fold the [1,1] half of the row FIR into the conv weights:
    # w'[cin, i'*3+q, cout], i' in 0..3: w'0=w0, w'1=w0+w1, w'2=w1+w2, w'3=w2
    wt = weight.transpose(1, 2, 3, 0) * np.float32(W_LRMUL * SQRT2)  # [cin,3,3,cout]
    wrow = np.stack(
        [wt[:, 0], wt[:, 0] + wt[:, 1], wt[:, 1] + wt[:, 2], wt[:, 2]], axis=1
    )  # [cin, 4, 3, cout]
    w_host = np.ascontiguousarray(wrow.reshape(CIN, 12, COUT).astype(np.float16))
    b_host = np.ascontiguousarray(
        (bias * np.float32(SQRT2)).astype(np.float32).reshape(4, 128).T
    )

    in_maps = []
    for c in range(NCORES):
        sl = slice(c * NPC, (c + 1) * NPC)
        in_maps.append(
            {
                "xe": np.ascontiguousarray(xe_dev[sl]),
                "xes": np.ascontiguousarray(xes_dev[sl]),
                "xo": np.ascontiguousarray(xo_dev[sl]),
                "xos": np.ascontiguousarray(xos_dev[sl]),
                "w": w_host,
                "b": b_host,
            }
        )
    return in_maps


def run(in_maps, trace=False, **kw):
    nc = get_nc()
    return run_bass_kernel_spmd(nc, in_maps, list(range(NCORES)), trace=trace, **kw)


def kernel(x, weight, bias, fir):
    res = run(prep_inputs(x, weight, bias, fir)).results
    out = np.concatenate([r["out"] for r in res], axis=0)
    return out.astype(np.float32)


# revision 39
# speedup vs baseline: 1.4785x; 1.4785x over previous
"""Trainium2 Bass kernel for ConvDownsample2d (FIR blur + 3x3/s2 conv + bias + leaky_relu*sqrt2).

Contract: kernel(**inputs) takes FULL inputs (x[16,512,64,64] f32, weight[512,512,3,3],
bias[512], fir[4,4]) and returns the FULL output [16,512,32,32] f32.

Strategy (hardcoded for this problem size):
  - Data-parallel over batch: 16 images / 8 cores = 2 images per core. No collectives.
  - Column-phase decomposition: host supplies 4 fp16 plane tensors per image
    (even/odd columns of x, each also at a one-element-shifted SBUF offset) so
    that (a) every VectorE blur operand is 4B-aligned and (b) every TensorE rhs
    has a PACKED last dimension. HW-measured matmul rates: packed-cols rhs
    ~129 ns/MM vs ~201 ns/MM for the naive stride-2 rhs at N=512 fp16.
  - Blur: z_e[v] = (x_e[v-1]+x_o[v]) + 3(x_o[v-1]+x_e[v]),
          z_o[v] = (x_o[v-1]+x_e[v+1]) + 3(x_e[v]+x_o[v]) per column phase,
    then the row blur y_p[r] = (z_p[r-2]+z_p[r+1]) + 3(z_p[r-1]+z_p[r]).
    Pairwise adds on VectorE (fp16 2x mode), the x3 via ScalarE scaled copies
    (GpSimd offload was measured slower on HW and is not used).
  - Conv: accumulated 128x128x512 fp16 matmuls (channels on partitions, 9 taps x
    4 cin-chunks into PSUM); rhs = y_phase[rows strided, 32 packed cols];
    mc-outer loop order staggers PSUM completion so the Prelu epilogue drains
    overlap the next output-chunk's matmuls.
  - Epilogue: single ScalarE Prelu (bias + leaky_relu(0.2)) per PSUM tile,
    DMA out f32 (sqrt2 folded into weights+bias on host).
"""

import sys

for p in ("/opt/trn_rl_repo", "/opt/pypackages"):
    if p not in sys.path:
        sys.path.insert(0, p)

import numpy as np
from contextlib import ExitStack

from concourse import bass, bacc, mybir, tile
from concourse.bass_utils import run_bass_kernel_spmd

F16 = mybir.dt.float16
F32 = mybir.dt.float32

NCORES = 8
NPC = 2            # images per core
N_TOT = 16         # total batch
CIN = 512
COUT = 512
H = W = 64
OH = OW = 32
KS = 3
W_LRMUL = 1.0 / np.sqrt(CIN * COUT * KS * KS)
SQRT2 = np.sqrt(2.0)

MT = 4             # top margin of z/y tiles (rows)
PW = 36            # plane width (cols): interior v=0..31 at cols 2..33
XB = 2             # x-plane prefetch buffers
YB = 3             # y buffers

USE_PRELU = True   # fused ScalarE Prelu epilogue (not implemented in CoreSim;
                   # validated end-to-end on HW via the rel-err check)

_CACHE = {}


def _build(reps=1):
    nc = bacc.Bacc("TRN2", target_bir_lowering=False, debug=False, enable_asserts=False)

    xe_d = nc.dram_tensor("xe", [NPC, CIN, H, PW], F16, kind="ExternalInput")
    xes_d = nc.dram_tensor("xes", [NPC, CIN, H, PW], F16, kind="ExternalInput")
    xo_d = nc.dram_tensor("xo", [NPC, CIN, H, PW], F16, kind="ExternalInput")
    xos_d = nc.dram_tensor("xos", [NPC, CIN, H, PW], F16, kind="ExternalInput")
    w_d = nc.dram_tensor("w", [CIN, 9, COUT], F16, kind="ExternalInput")
    b_d = nc.dram_tensor("b", [128, 4], F32, kind="ExternalInput")
    o_d = nc.dram_tensor("out", [NPC, COUT, OH, OW], F32, kind="ExternalOutput")

    AL = mybir.AluOpType
    AF = mybir.ActivationFunctionType

    with tile.TileContext(nc) as tc, ExitStack() as ctx:
        cpool = ctx.enter_context(tc.tile_pool(name="const", bufs=1))
        bpool = ctx.enter_context(tc.tile_pool(name="blur", bufs=1))
        opool = ctx.enter_context(tc.tile_pool(name="outp", bufs=8))
        ppool = ctx.enter_context(
            tc.tile_pool(name="psum", bufs=1, space=bass.MemorySpace.PSUM)
        )

        # --- constants (kc=0 weights first so the first matmuls are not stuck
        # behind the full weight transfer) ---
        w_sb = cpool.tile([128, 4, 9, COUT], F16, name="w_sb")
        nc.sync.dma_start(out=w_sb[:, 0], in_=w_d[0:128])
        b_sb = cpool.tile([128, 4], F32, name="b_sb")
        nc.sync.dma_start(out=b_sb[:], in_=b_d[:])

        # --- blur tiles ---
        def planes(name, n):
            return [bpool.tile([128, H, PW], F16, name=f"{name}{i}") for i in range(n)]

        xe = planes("xe", XB)
        xes = planes("xes", XB)
        xo = planes("xo", XB)
        xos = planes("xos", XB)
        ta = planes("ta", 2)     # Pool-written, DVE-read
        tb = planes("tb", 2)     # DVE-written, ACT-read
        b3 = planes("b3", 2)     # ACT-written, DVE-read
        t1 = planes("t1", 1)[0]
        t2 = planes("t2", 2)
        t23 = planes("t23", 2)
        ze = bpool.tile([128, 70, PW], F16, name="ze")
        zo = bpool.tile([128, 70, PW], F16, name="zo")
        ye = [bpool.tile([128, 70, PW], F16, name=f"ye{i}") for i in range(YB)]
        yo = [bpool.tile([128, 70, PW], F16, name=f"yo{i}") for i in range(YB)]

        # --- zero guards once; per-chunk writes stay in the interior ---
        for z_ in (ze, zo):
            nc.vector.memzero(z_[:, 2:4, 2:34])            # z rows -2, -1
            nc.vector.memzero(z_[:, 68:69, 2:34])          # z row 64
        for y_ in ye + yo:
            nc.vector.memzero(y_[:, 3:4, 0:PW])            # y row -1
        for y_ in yo:
            nc.vector.memzero(y_[:, 4:68, 0:2])            # y_o col -1 (at col 1)

        n_imgs = reps * NPC
        chunks = [(i % NPC, kc) for i in range(n_imgs) for kc in range(4)]

        def dma_planes(g):
            n, kc = chunks[g]
            sl = slice(kc * 128, (kc + 1) * 128)
            i = g % XB
            nc.sync.dma_start(out=xe[i][:], in_=xe_d[n, sl])
            nc.sync.dma_start(out=xo[i][:], in_=xo_d[n, sl])
            nc.sync.dma_start(out=xes[i][:], in_=xes_d[n, sl])
            nc.sync.dma_start(out=xos[i][:], in_=xos_d[n, sl])

        for g in range(min(XB, len(chunks))):
            dma_planes(g)
        for kc in range(1, 4):
            nc.sync.dma_start(out=w_sb[:, kc], in_=w_d[kc * 128:(kc + 1) * 128])

        C = slice(2, 34)       # interior plane cols (v = col-2)
        for g, (n, kc) in enumerate(chunks):
            i = g % XB
            xe_, xes_, xo_, xos_ = xe[i], xes[i], xo[i], xos[i]
            ta_, tb_, b3_ = ta[g % 2], tb[g % 2], b3[g % 2]
            ye_, yo_ = ye[g % YB], yo[g % YB]

            # ---- W-blur (column phases) ----
            # z_e = (x_e[v-1] + x_o[v]) + 3*(x_o[v-1] + x_e[v])
            nc.vector.tensor_tensor(ta_[:, :, C], xes_[:, :, C], xo_[:, :, C], AL.add)
            nc.vector.tensor_tensor(tb_[:, :, C], xos_[:, :, C], xe_[:, :, C], AL.add)
            nc.scalar.activation(b3_[:, :, C], tb_[:, :, C], AF.Identity, scale=3.0)
            nc.vector.tensor_tensor(
                ze[:, MT:MT + 64, C], ta_[:, :, C], b3_[:, :, C], AL.add
            )
            # z_o = (x_o[v-1] + x_e[v+1]) + 3*(x_e[v] + x_o[v])
            ta2_, tb2_, b32_ = ta[(g + 1) % 2], tb[(g + 1) % 2], b3[(g + 1) % 2]
            nc.vector.tensor_tensor(
                ta2_[:, :, C], xos_[:, :, C], xes_[:, :, 4:36], AL.add
            )
            nc.vector.tensor_tensor(tb2_[:, :, C], xe_[:, :, C], xo_[:, :, C], AL.add)
            nc.scalar.activation(b32_[:, :, C], tb2_[:, :, C], AF.Identity, scale=3.0)
            nc.vector.tensor_tensor(
                zo[:, MT:MT + 64, C], ta2_[:, :, C], b32_[:, :, C], AL.add
            )
            # prefetch AFTER the last reader of this chunk's x planes
            if g + XB < len(chunks):
                dma_planes(g + XB)

            # ---- H-blur per column phase ----
            # y_p[r] = (z_p[r-2] + z_p[r+1]) + 3*(z_p[r-1] + z_p[r])
            for pb, (z_, y_) in enumerate(((ze, ye_), (zo, yo_))):
                t2_, t23_ = t2[pb], t23[pb]
                nc.vector.tensor_tensor(
                    t1[:, :, C], z_[:, 2:66, C], z_[:, 5:69, C], AL.add
                )
                nc.vector.tensor_tensor(
                    t2_[:, :, C], z_[:, 3:67, C], z_[:, 4:68, C], AL.add
                )
                nc.scalar.activation(t23_[:, :, C], t2_[:, :, C], AF.Identity, scale=3.0)
                nc.vector.tensor_tensor(
                    y_[:, MT:MT + 64, C], t1[:, :, C], t23_[:, :, C], AL.add
                )

            # ---- conv taps: psum[mc][uh] += w[p,q,kc,mc].T @ y[2u+p-1, 2v+q-1] ----
            if kc == 0:
                psum = [
                    [
                        ppool.tile([128, 16, OW], F32, tag=f"ps{mc}{uh}", name=f"ps{mc}{uh}")
                        for uh in range(2)
                    ]
                    for mc in range(4)
                ]
            for mc in range(4):
                for pq in range(9):
                    p, q = divmod(pq, 3)
                    lhsT = w_sb[:, kc, pq, mc * 128:(mc + 1) * 128]
                    plane = ye_ if q == 1 else yo_
                    t0 = 1 if q == 0 else 2
                    for uh in range(2):
                        r0 = MT - 1 + p + 32 * uh
                        rhs = plane[:, r0:r0 + 32:2, t0:t0 + 32]
                        nc.tensor.matmul(
                            psum[mc][uh][:],
                            lhsT,
                            rhs,
                            start=(kc == 0 and pq == 0),
                            stop=(kc == 3 and pq == 8),
                        )
                if kc == 3:
                    # epilogue for this mc: overlaps the next mc's matmuls
                    with tc.high_priority():
                        for uh in range(2):
                            ob = opool.tile([128, 16, OW], F32, tag="ob", name="ob")
                            if USE_PRELU:
                                nc.scalar.activation(
                                    ob[:], psum[mc][uh][:], AF.Prelu,
                                    bias=b_sb[:, mc:mc + 1], scale=1.0, alpha=0.2,
                                )
                            else:
                                tbo = opool.tile([128, 16, OW], F32, tag="tbo", name="tbo")
                                nc.scalar.activation(
                                    tbo[:], psum[mc][uh][:], AF.Identity,
                                    bias=b_sb[:, mc:mc + 1], scale=1.0,
                                )
                                nc.vector.scalar_tensor_tensor(
                                    ob[:], tbo[:], 0.2, tbo[:], AL.mult, AL.max
                                )
                            nc.sync.dma_start(
                                out=o_d[n, mc * 128:(mc + 1) * 128, uh * 16:(uh + 1) * 16, :],
                                in_=ob[:],
                            )

    nc.compile()
    return nc


def get_nc(reps=1):
    key = f"nc{reps}"
    if key not in _CACHE:
        _CACHE[key] = _build(reps)
    return _CACHE[key]


def prep_inputs(x, weight, bias, fir):
    """Host-side shard + fold constants + phase-plane layout. Returns per-core maps."""
    x = np.asarray(x, dtype=np.float32)
    weight = np.asarray(weight, dtype=np.float32)
    bias = np.asarray(bias, dtype=np.float32)
    fir = np.asarray(fir, dtype=np.float32)

    # normalized separable fir = fir[0,0] * outer([1,3,3,1],[1,3,3,1]);
    # fold fir[0,0] into x, integer taps run on device.
    scale = float(fir[0, 0])
    x16 = (x * scale).astype(np.float16)
    x_e = x16[:, :, :, 0::2]   # [N, C, 64, 32]
    x_o = x16[:, :, :, 1::2]

    # plane layouts (width PW=36): col t <-> v = t-2 (normal) / v = t-3 (shifted);
    # guard zeros baked in.
    def lay(plane, off):
        buf = np.zeros((N_TOT, CIN, H, PW), dtype=np.float16)
        buf[:, :, :, off:off + 32] = plane
        return buf

    xe_dev = lay(x_e, 2)
    xes_dev = lay(x_e, 3)
    xo_dev = lay(x_o, 2)
    xos_dev = lay(x_o, 3)

    # w_host[cin, p*3+q, cout] = weight[cout, cin, p, q] * W_LRMUL * sqrt2
    w_host = np.ascontiguousarray(
        (weight.transpose(1, 2, 3, 0) * np.float32(W_LRMUL * SQRT2))
        .reshape(CIN, 9, COUT)
        .astype(np.float16)
    )
    b_host = np.ascontiguousarray(
        (bias * np.float32(SQRT2)).astype(np.float32).reshape(4, 128).T
    )

    in_maps = []
    for c in range(NCORES):
        sl = slice(c * NPC, (c + 1) * NPC)
        in_maps.append(
            {
                "xe": np.ascontiguousarray(xe_dev[sl]),
                "xes": np.ascontiguousarray(xes_dev[sl]),
                "xo": np.ascontiguousarray(xo_dev[sl]),
                "xos": np.ascontiguousarray(xos_dev[sl]),
                "w": w_host,
                "b": b_host,
            }
        )
    return in_maps


def run(in_maps, trace=False, **kw):
    nc = get_nc()
    return run_bass_kernel_spmd(nc, in_maps, list(range(NCORES)), trace=trace, **kw)


def kernel(x, weight, bias, fir):
    res = run(prep_inputs(x, weight, bias, fir)).results
    out = np.concatenate([r["out"] for r in res], axis=0)
    return out.astype(np.float32)
